# revision 16
# baseline (speedup 1.0000x reference)
"""Distributed Trainium2 kernel for nn_Attention_64854006169830.

Strategy (8 NeuronCores, SPMD):
  - QKV + attention: head-parallel (core i computes head i for all B*L rows),
    with activations kept feature-major ("transposed") so every matmul uses
    natural weight layouts. Softmax is computed on transposed scores
    (keys on partitions): exp on ACT, row-sums via ones-matmul on PE,
    normalization after PE-transpose back to row-major.
  - ctx redistribution head-shard -> row-shard via per-batch AllToAll.
  - LN + FF + collapse(d->1): row-parallel (core i owns 256 L-positions of
    each batch).
  - c = [B, L] gathered with a tiny AllGather; the final two small matmuls
    (L->L, L->OUT) run redundantly on every core (weights replicated).
Compute dtype: bf16 (f32 accumulation in PSUM); verified ~0.6% rel err.
"""
import sys
import math

for _p in ("/opt/trn_rl_repo", "/opt/trn_rl_repo/concourse"):
    if _p not in sys.path:
        sys.path.insert(0, _p)

import numpy as np
import ml_dtypes

B, L, D, H, OUT = 4, 2048, 1024, 8, 256
DH = D // H          # 128
N = B * L            # 8192 rows
NC = 8               # cores
RPC = N // NC        # 1024 rows per core (as 4 batches x 256 L-positions)
LPC = L // NC        # 256 L-positions per core per batch
EPS = 1e-12

_CACHE = {}


def _build_nc():
    import concourse.bass as bass
    import concourse.tile as tile
    from concourse import bacc, mybir
    from concourse.masks import make_identity

    BF = mybir.dt.bfloat16
    F32 = mybir.dt.float32
    AF = mybir.ActivationFunctionType
    OP = mybir.AluOpType

    nc = bacc.Bacc("TRN2", debug=False, num_devices=NC)

    # ---- parameters (per-core values supplied via in_maps) ----
    xbf = nc.dram_tensor("xbf", [N, D], BF, kind="ExternalInput")
    xrows = nc.dram_tensor("xrows", [RPC, D], BF, kind="ExternalInput")
    wqkv = nc.dram_tensor("wqkv", [3, D, DH], BF, kind="ExternalInput")
    bqkv = nc.dram_tensor("bqkv", [3, DH], F32, kind="ExternalInput")
    wff = nc.dram_tensor("wff", [D, D], BF, kind="ExternalInput")
    bff = nc.dram_tensor("bff", [D], F32, kind="ExternalInput")
    gamma = nc.dram_tensor("gamma", [D], F32, kind="ExternalInput")
    beta = nc.dram_tensor("beta", [D], F32, kind="ExternalInput")
    wc1 = nc.dram_tensor("wc1", [D, D], BF, kind="ExternalInput")   # gamma-folded
    bc1 = nc.dram_tensor("bc1", [D], F32, kind="ExternalInput")     # beta-folded
    wc2 = nc.dram_tensor("wc2", [D], BF, kind="ExternalInput")
    bc2 = nc.dram_tensor("bc2", [1], F32, kind="ExternalInput")
    wl1 = nc.dram_tensor("wl1", [L, L], BF, kind="ExternalInput")
    bl1 = nc.dram_tensor("bl1", [L], F32, kind="ExternalInput")
    wl2 = nc.dram_tensor("wl2", [L, OUT], BF, kind="ExternalInput")
    bl2 = nc.dram_tensor("bl2", [OUT], F32, kind="ExternalInput")
    out = nc.dram_tensor("out", [B, OUT], F32, kind="ExternalOutput")

    # ---- internal DRAM ----
    a2a_in = nc.dram_tensor("a2a_in", [N, DH], BF)
    a2a_out = nc.dram_tensor("a2a_out", [N, DH], BF)
    sums_hbm = nc.dram_tensor("sums_hbm", [N], F32)
    c_ag_in = nc.dram_tensor("c_ag_in", [RPC], F32)
    c_ag_out = nc.dram_tensor("c_ag_out", [N], F32, addr_space="Shared")

    def bcast(dram_handle, parts, free):
        """Broadcast a [free] DRAM vector across `parts` partitions."""
        ap = dram_handle.ap()
        return bass.AP(tensor=ap.tensor, offset=0, ap=[[0, parts], [1, free]])

    RG = [list(range(NC))]
    ISQ = 1.0  # 1/sqrt(DH) folded into wq/bq on host

    from contextlib import ExitStack

    with tile.TileContext(nc) as tc, ExitStack() as root:
        glob = root.enter_context(tc.tile_pool(name="glob", bufs=1))
        ident = glob.tile([128, 128], BF)
        make_identity(nc, ident[:])
        ones128 = glob.tile([128, 1], BF)
        nc.vector.memset(ones128[:], 1.0)
        eps_sb = glob.tile([128, 1], F32)
        nc.vector.memset(eps_sb[:], EPS)

        phAB = root.enter_context(ExitStack())
        qkv_pool = phAB.enter_context(tc.tile_pool(name="qkv", bufs=1))
        # persistent through phases A+B
        qkvT = qkv_pool.tile([128, 3, N], BF)   # 48KB/part: q^T, k^T, v^T

        # ================= Phase A: QKV^T =================
        with ExitStack() as phA:
            xt_pool = phA.enter_context(tc.tile_pool(name="xt", bufs=1))
            wq_pool = phA.enter_context(tc.tile_pool(name="wqkv", bufs=1))
            psA = phA.enter_context(tc.tile_pool(name="psA", bufs=4, space="PSUM"))

            xt = xt_pool.tile([128, D // 128, N], BF)   # x^T, 128KB/part
            for kc in range(D // 128):
                nc.sync.dma_start(out=xt[:, kc, :],
                                  in_=xbf.ap()[:, kc * 128:(kc + 1) * 128],
                                  transpose=True)
            wq_sb = wq_pool.tile([128, 3, D // 128, DH], BF)
            nc.sync.dma_start(
                out=wq_sb[:],
                in_=bass.AP(tensor=wqkv.ap().tensor, offset=0,
                            ap=[[DH, 128], [D * DH, 3], [128 * DH, D // 128], [1, DH]]))
            bq_sb = wq_pool.tile([128, 3], F32)
            nc.sync.dma_start(
                out=bq_sb[:],
                in_=bass.AP(tensor=bqkv.ap().tensor, offset=0,
                            ap=[[1, 128], [DH, 3]]))

            NRC = N // 512  # 16 chunks of 512 rows
            for s in range(3):
                for g in range(NRC // 4):
                    pst = [psA.tile([128, 512], F32, tag="qkvps", name=f"qkvps{s}_{g}_{j}")
                           for j in range(4)]
                    for kc in range(D // 128):
                        for r4 in range(4):
                            rc = g * 4 + r4
                            nc.tensor.matmul(
                                pst[r4][:], wq_sb[:, s, kc, :],
                                xt[:, kc, rc * 512:(rc + 1) * 512],
                                start=(kc == 0), stop=(kc == D // 128 - 1))
                    for r4 in range(4):
                        rc = g * 4 + r4
                        nc.scalar.activation(
                            qkvT[:, s, rc * 512:(rc + 1) * 512], pst[r4][:],
                            AF.Identity, bias=bq_sb[:, s:s + 1], scale=1.0)

        # ================= Phase B: attention per batch =================
        with ExitStack() as phB:
            vnat_pool = phB.enter_context(tc.tile_pool(name="vnat", bufs=1))
            pT_pool = phB.enter_context(tc.tile_pool(name="pT", bufs=2))
            ctxT_pool = phB.enter_context(tc.tile_pool(name="ctxT", bufs=2))
            sums_pool = phB.enter_context(tc.tile_pool(name="sums", bufs=1))
            recip_pool = phB.enter_context(tc.tile_pool(name="recip", bufs=2))
            norm_pool = phB.enter_context(tc.tile_pool(name="norm", bufs=3))
            psS = phB.enter_context(tc.tile_pool(name="psS", bufs=2, space="PSUM"))
            psC = phB.enter_context(tc.tile_pool(name="psC", bufs=2, space="PSUM"))
            psSum = phB.enter_context(tc.tile_pool(name="psSum", bufs=2, space="PSUM"))
            psTrB = phB.enter_context(tc.tile_pool(name="psTrB", bufs=2, space="PSUM"))

            sums_sb = sums_pool.tile([1, N], F32)
            KCB = L // 128  # 16 key chunks per batch

            # v row-major via PE transpose
            vnat = vnat_pool.tile([128, N // 128, DH], BF)  # 16KB/part
            for rc in range(N // 128):
                tps = psTrB.tile([128, 128], BF, tag="ctr")
                nc.tensor.transpose(tps[:], qkvT[:, 2, rc * 128:(rc + 1) * 128], ident[:])
                nc.vector.tensor_copy(vnat[:, rc, :], tps[:])

            for b in range(B):
                ctxT_sb = ctxT_pool.tile([128, L], BF, tag="ctxT")
                for qc in range(L // 512):
                    pT = pT_pool.tile([128, KCB, 512], BF, tag="pT")
                    q_sl = qkvT[:, 0, b * L + qc * 512: b * L + (qc + 1) * 512]
                    for kc in range(KCB):
                        sps = psS.tile([128, 512], F32, tag="sps")
                        nc.tensor.matmul(
                            sps[:],
                            qkvT[:, 1, b * L + kc * 128: b * L + (kc + 1) * 128],
                            q_sl, start=True, stop=True)
                        nc.scalar.activation(pT[:, kc, :], sps[:], AF.Exp, scale=ISQ)
                    cps = psC.tile([128, 512], F32, tag="cps")
                    sps2 = psSum.tile([1, 512], F32, tag="sps2")
                    for kc in range(KCB):
                        nc.tensor.matmul(cps[:], vnat[:, b * KCB + kc, :], pT[:, kc, :],
                                         start=(kc == 0), stop=(kc == KCB - 1))
                    for kc in range(KCB):
                        nc.tensor.matmul(sps2[:], ones128[:], pT[:, kc, :],
                                         start=(kc == 0), stop=(kc == KCB - 1))
                    nc.vector.tensor_copy(ctxT_sb[:, qc * 512:(qc + 1) * 512], cps[:])
                    nc.vector.tensor_copy(sums_sb[:, b * L + qc * 512: b * L + (qc + 1) * 512],
                                          sps2[:])
                # reciprocal of sums, transposed into per-row columns
                nc.sync.dma_start(out=sums_hbm.ap()[b * L:(b + 1) * L],
                                  in_=sums_sb[0:1, b * L:(b + 1) * L])
                rraw = recip_pool.tile([128, KCB], F32, tag="rraw")
                nc.sync.dma_start(
                    out=rraw[:],
                    in_=sums_hbm.ap()[b * L:(b + 1) * L].rearrange("(j p) -> p j", p=128))
                rcols = recip_pool.tile([128, KCB], F32, tag="rcols")
                nc.vector.reciprocal(rcols[:], rraw[:])
                # transpose ctx^T back to row-major, normalize, store to a2a_in
                for rc in range(KCB):
                    tps = psTrB.tile([128, 128], BF, tag="ctr")
                    nc.tensor.transpose(tps[:], ctxT_sb[:, rc * 128:(rc + 1) * 128], ident[:])
                    nrm = norm_pool.tile([128, DH], BF, tag="nrm")
                    nc.vector.tensor_scalar_mul(nrm[:], tps[:], rcols[:, rc:rc + 1])
                    nc.sync.dma_start(
                        out=a2a_in.ap()[b * L + rc * 128: b * L + (rc + 1) * 128, :],
                        in_=nrm[:])
                nc.gpsimd.collective_compute(
                    "AllToAll", OP.bypass,
                    ins=[a2a_in.ap()[b * L:(b + 1) * L, :]],
                    outs=[a2a_out.ap()[b * L:(b + 1) * L, :]],
                    replica_groups=RG)
        phAB.close()  # release qkvT before Phase C

        # ================= Phase C: row-parallel LN/FF/collapse =================
        with ExitStack() as phC:
            wC_pool = phC.enter_context(tc.tile_pool(name="wC", bufs=1))
            rowC = phC.enter_context(tc.tile_pool(name="rowC", bufs=2))
            h2T_pool = phC.enter_context(tc.tile_pool(name="h2T", bufs=1))
            psFF = phC.enter_context(tc.tile_pool(name="psFF", bufs=2, space="PSUM"))
            psTrC = phC.enter_context(tc.tile_pool(name="psTrC", bufs=2, space="PSUM"))
            psC1 = phC.enter_context(tc.tile_pool(name="psC1", bufs=2, space="PSUM"))
            psC2 = phC.enter_context(tc.tile_pool(name="psC2", bufs=1, space="PSUM"))
            psFin = phC.enter_context(tc.tile_pool(name="psFin", bufs=1, space="PSUM"))

            DKC = D // 128  # 8
            wff_sb = wC_pool.tile([128, DKC, D], BF)
            nc.sync.dma_start(
                out=wff_sb[:],
                in_=bass.AP(tensor=wff.ap().tensor, offset=0,
                            ap=[[D, 128], [128 * D, DKC], [1, D]]))
            wc1_sb = wC_pool.tile([128, DKC, D], BF)
            nc.sync.dma_start(
                out=wc1_sb[:],
                in_=bass.AP(tensor=wc1.ap().tensor, offset=0,
                            ap=[[D, 128], [128 * D, DKC], [1, D]]))
            wc2_sb = wC_pool.tile([128, DKC], BF)
            nc.sync.dma_start(
                out=wc2_sb[:],
                in_=bass.AP(tensor=wc2.ap().tensor, offset=0,
                            ap=[[1, 128], [128, DKC]]))
            bc1_sb = wC_pool.tile([128, DKC], F32)
            nc.sync.dma_start(
                out=bc1_sb[:],
                in_=bass.AP(tensor=bc1.ap().tensor, offset=0,
                            ap=[[1, 128], [128, DKC]]))
            bc2_sb = wC_pool.tile([1, 1], F32)
            nc.sync.dma_start(out=bc2_sb[:], in_=bc2.ap())
            gamma_bc = wC_pool.tile([128, D], F32)
            nc.sync.dma_start(out=gamma_bc[:], in_=bcast(gamma, 128, D))
            beta_bc = wC_pool.tile([128, D], F32)
            nc.sync.dma_start(out=beta_bc[:], in_=bcast(beta, 128, D))
            bff_bc = wC_pool.tile([128, D], F32)
            nc.sync.dma_start(out=bff_bc[:], in_=bcast(bff, 128, D))
            # large final weights (wl1 loaded on demand per 512-col chunk)
            wl1_pool = phC.enter_context(tc.tile_pool(name="wl1", bufs=2))
            wl2_sb = wC_pool.tile([128, L // 128, OUT], BF)
            nc.sync.dma_start(
                out=wl2_sb[:],
                in_=bass.AP(tensor=wl2.ap().tensor, offset=0,
                            ap=[[OUT, 128], [128 * OUT, L // 128], [1, OUT]]))

            h2T_all = h2T_pool.tile([128, DKC, RPC], BF)
            c2_sb = h2T_pool.tile([1, RPC], F32)

            def layernorm_rows(src_f32, dst, apply_gb):
                """src [128, D] f32 -> dst (normalized; optionally *gamma+beta)."""
                stats = rowC.tile([128, 2, nc.vector.BN_STATS_DIM], F32, tag="stats")
                for sg in range(2):
                    nc.vector.bn_stats(stats[:, sg, :], src_f32[:, sg * 512:(sg + 1) * 512])
                mv = rowC.tile([128, nc.vector.BN_AGGR_DIM], F32, tag="mv")
                nc.vector.bn_aggr(mv[:], stats[:])
                sq = rowC.tile([128, 1], F32, tag="sq")
                nc.scalar.activation(sq[:], mv[:, 1:2], AF.Sqrt, bias=eps_sb[:], scale=1.0)
                rstd = rowC.tile([128, 1], F32, tag="rstd")
                nc.vector.reciprocal(rstd[:], sq[:])
                if apply_gb:
                    z = rowC.tile([128, D], F32, tag="zf")
                    nc.vector.tensor_scalar(z[:], src_f32[:], mv[:, 0:1], rstd[:],
                                            op0=OP.subtract, op1=OP.mult)
                    zg = rowC.tile([128, D], F32, tag="zg")
                    nc.vector.tensor_mul(zg[:], z[:], gamma_bc[:])
                    nc.vector.tensor_add(dst[:], zg[:], beta_bc[:])
                else:
                    nc.vector.tensor_scalar(dst[:], src_f32[:], mv[:, 0:1], rstd[:],
                                            op0=OP.subtract, op1=OP.mult)

            for t in range(RPC // 128):
                b, e = t // 2, t % 2
                # rows of this tile: batch b, L-positions [i*LPC + e*128 + p)
                # (core-specific x rows arrive pre-sliced via `xrows`)
                ctx_t = rowC.tile([128, H, DH], BF, tag="ctx_t")
                nc.sync.dma_start(
                    out=ctx_t[:],
                    in_=bass.AP(tensor=a2a_out.ap().tensor,
                                offset=(b * L + e * 128) * DH,
                                ap=[[DH, 128], [LPC * DH, H], [1, DH]]))
                x_t = rowC.tile([128, D], BF, tag="x_t")
                nc.sync.dma_start(out=x_t[:], in_=xrows.ap()[t * 128:(t + 1) * 128, :])
                s_t = rowC.tile([128, D], F32, tag="s_t")
                nc.vector.tensor_add(s_t[:], x_t[:], ctx_t[:].rearrange("p h d -> p (h d)"))
                h1b = rowC.tile([128, D], BF, tag="h1b")
                layernorm_rows(s_t, h1b, apply_gb=True)
                # h1^T for the ff matmul
                h1T = rowC.tile([128, DKC, 128], BF, tag="h1T")
                for kc in range(DKC):
                    tps = psTrC.tile([128, 128], BF, tag="htr")
                    nc.tensor.transpose(tps[:], h1b[:, kc * 128:(kc + 1) * 128], ident[:])
                    nc.vector.tensor_copy(h1T[:, kc, :], tps[:])
                # ff natural [128 rows, D]
                f_t = rowC.tile([128, D], BF, tag="f_t")
                for dc in range(2):
                    fps = psFF.tile([128, 512], F32, tag="fps")
                    for kc in range(DKC):
                        nc.tensor.matmul(fps[:], h1T[:, kc, :],
                                         wff_sb[:, kc, dc * 512:(dc + 1) * 512],
                                         start=(kc == 0), stop=(kc == DKC - 1))
                    tmp = rowC.tile([128, 512], F32, tag="fftmp")
                    nc.vector.tensor_add(tmp[:], fps[:], bff_bc[:, dc * 512:(dc + 1) * 512])
                    nc.vector.tensor_scalar_max(f_t[:, dc * 512:(dc + 1) * 512], tmp[:], 0.0)
                s2_t = rowC.tile([128, D], F32, tag="s2_t")
                nc.vector.tensor_add(s2_t[:], h1b[:], f_t[:])
                h2b = rowC.tile([128, D], BF, tag="h2b")
                layernorm_rows(s2_t, h2b, apply_gb=False)  # gamma/beta folded into wc1
                for kc in range(DKC):
                    tps = psTrC.tile([128, 128], BF, tag="htr")
                    nc.tensor.transpose(tps[:], h2b[:, kc * 128:(kc + 1) * 128], ident[:])
                    nc.vector.tensor_copy(h2T_all[:, kc, t * 128:(t + 1) * 128], tps[:])

            # c1^T = relu(wc1'^T h2 + bc1') ; then c2 = relu(c1 @ wc2 + bc2)
            c1T = h2T_pool.tile([128, DKC, RPC], BF)
            for fc in range(DKC):
                for rc in range(RPC // 512):
                    cps = psC1.tile([128, 512], F32, tag="c1ps")
                    for kc in range(DKC):
                        nc.tensor.matmul(cps[:], wc1_sb[:, kc, fc * 128:(fc + 1) * 128],
                                         h2T_all[:, kc, rc * 512:(rc + 1) * 512],
                                         start=(kc == 0), stop=(kc == DKC - 1))
                    nc.scalar.activation(c1T[:, fc, rc * 512:(rc + 1) * 512], cps[:],
                                         AF.Relu, bias=bc1_sb[:, fc:fc + 1], scale=1.0)
            for rc in range(RPC // 512):
                c2ps = psC2.tile([1, 512], F32, tag="c2ps")
                for kc in range(DKC):
                    nc.tensor.matmul(c2ps[:], wc2_sb[:, kc:kc + 1],
                                     c1T[:, kc, rc * 512:(rc + 1) * 512],
                                     start=(kc == 0), stop=(kc == DKC - 1))
                nc.scalar.activation(c2_sb[0:1, rc * 512:(rc + 1) * 512], c2ps[:],
                                     AF.Relu, bias=bc2_sb[0:1, :], scale=1.0)
            nc.sync.dma_start(out=c_ag_in.ap().rearrange("(o n) -> o n", o=1),
                              in_=c2_sb[0:1, :])
            nc.gpsimd.collective_compute(
                "AllGather", OP.bypass,
                ins=[c_ag_in.ap()], outs=[c_ag_out.ap()], replica_groups=RG)

            # final: out = relu(c @ wl1 + bl1) @ wl2 + bl2 (redundant on all cores)
            cT = rowC.tile([128, 2, NC, B], F32, tag="cT")
            for e in range(2):
                nc.sync.dma_start(
                    out=cT[:, e, :, :],
                    in_=bass.AP(tensor=c_ag_out.ap().tensor, offset=e * 128,
                                ap=[[1, 128], [RPC, NC], [LPC, B]]))
            cTb = rowC.tile([128, 2, NC, B], BF, tag="cTb")
            nc.vector.tensor_copy(cTb[:], cT[:])
            bl1_bc = wC_pool.tile([B, L], F32)
            nc.sync.dma_start(out=bl1_bc[:], in_=bcast(bl1, B, L))
            bl2_bc = wC_pool.tile([B, OUT], F32)
            nc.sync.dma_start(out=bl2_bc[:], in_=bcast(bl2, B, OUT))
            c1f = rowC.tile([B, L], BF, tag="c1f")
            for oc in range(L // 512):
                wl1_oc = wl1_pool.tile([128, L // 128, 512], BF, tag="wl1oc")
                nc.sync.dma_start(
                    out=wl1_oc[:],
                    in_=bass.AP(tensor=wl1.ap().tensor, offset=oc * 512,
                                ap=[[L, 128], [128 * L, L // 128], [1, 512]]))
                fps = psFin.tile([B, 512], F32, tag="finps")
                for kc in range(L // 128):
                    nc.tensor.matmul(fps[:], cTb[:, kc % 2, kc // 2, :],
                                     wl1_oc[:, kc, :],
                                     start=(kc == 0), stop=(kc == L // 128 - 1))
                tmp = rowC.tile([B, 512], F32, tag="fintmp")
                nc.vector.tensor_add(tmp[:], fps[:], bl1_bc[:, oc * 512:(oc + 1) * 512])
                nc.vector.tensor_scalar_max(c1f[:, oc * 512:(oc + 1) * 512], tmp[:], 0.0)
            c1fT = rowC.tile([128, L // 128, B], BF, tag="c1fT")
            for j in range(L // 128):
                tps = psTrC.tile([128, B], BF, tag="htr")
                nc.tensor.transpose(tps[:], c1f[0:B, j * 128:(j + 1) * 128], ident[0:B, 0:B])
                nc.vector.tensor_copy(c1fT[:, j, :], tps[:])
            ops = psFin.tile([B, OUT], F32, tag="finps")
            for kc in range(L // 128):
                nc.tensor.matmul(ops[:], c1fT[:, kc, :], wl2_sb[:, kc, :],
                                 start=(kc == 0), stop=(kc == L // 128 - 1))
            out_f = rowC.tile([B, OUT], F32, tag="out_f")
            nc.vector.tensor_add(out_f[:], ops[:], bl2_bc[:])
            nc.sync.dma_start(out=out.ap(), in_=out_f[:])

    nc.compile()
    return nc


def _to_bf16(a):
    return np.asarray(a, dtype=np.float32).astype(ml_dtypes.bfloat16)


def kernel(**inputs):
    from concourse.bass_utils import run_bass_kernel_spmd

    if "nc" not in _CACHE:
        _CACHE["nc"] = _build_nc()
    nc = _CACHE["nc"]

    x = np.asarray(inputs["x"], dtype=np.float32).reshape(N, D)
    isq = 1.0 / math.sqrt(DH)
    gamma_np = np.asarray(inputs["gamma"], dtype=np.float32)
    beta_np = np.asarray(inputs["beta"], dtype=np.float32)
    wc1_np = np.asarray(inputs["wc1"], dtype=np.float32)
    bc1_np = np.asarray(inputs["bc1"], dtype=np.float32)
    # fold LN2's gamma/beta into the c1 projection (h2 feeds only this matmul)
    wc1_f = gamma_np[:, None] * wc1_np
    bc1_f = bc1_np + beta_np @ wc1_np

    xbf = _to_bf16(x)
    shared = dict(
        xbf=xbf,
        wff=_to_bf16(inputs["wff"]),
        bff=np.asarray(inputs["bff"], np.float32),
        gamma=gamma_np, beta=beta_np,
        wc1=_to_bf16(wc1_f), bc1=bc1_f.astype(np.float32),
        wc2=_to_bf16(np.asarray(inputs["wc2"]).reshape(D)),
        bc2=np.asarray(inputs["bc2"], np.float32).reshape(1),
        wl1=_to_bf16(inputs["wl1"]), bl1=np.asarray(inputs["bl1"], np.float32),
        wl2=_to_bf16(inputs["wl2"]), bl2=np.asarray(inputs["bl2"], np.float32),
    )
    wq = np.asarray(inputs["wq"], np.float32) * isq
    bq = np.asarray(inputs["bq"], np.float32) * isq
    wk = np.asarray(inputs["wk"], np.float32)
    bk = np.asarray(inputs["bk"], np.float32)
    wv = np.asarray(inputs["wv"], np.float32)
    bv = np.asarray(inputs["bv"], np.float32)

    in_maps = []
    for i in range(NC):
        sl = slice(i * DH, (i + 1) * DH)
        wqkv_i = np.stack([wq[:, sl], wk[:, sl], wv[:, sl]])
        bqkv_i = np.stack([bq[sl], bk[sl], bv[sl]])
        # rows this core owns after the A2A: for each batch b, L-positions
        # [i*LPC, (i+1)*LPC) -> 8 row-tiles of 128 = (b, e) pairs
        xr = np.concatenate([
            x[b * L + i * LPC: b * L + (i + 1) * LPC, :] for b in range(B)
        ])  # [RPC, D] ordered (b, l-within-block)
        in_maps.append(dict(
            shared,
            wqkv=_to_bf16(wqkv_i),
            bqkv=bqkv_i.astype(np.float32),
            xrows=_to_bf16(xr),
        ))

    res = run_bass_kernel_spmd(nc, in_maps, core_ids=list(range(NC)))
    return np.asarray(res.results[0]["out"], dtype=np.float32)


# revision 26
# speedup vs baseline: 1.1049x; 1.1049x over previous
"""Distributed Trainium2 kernel for nn_Attention_64854006169830.

Strategy (8 NeuronCores, SPMD):
  - QKV + attention: head-parallel (core i computes head i for all B*L rows),
    with activations kept feature-major ("transposed") so every matmul uses
    natural weight layouts. Softmax is computed on transposed scores
    (keys on partitions): exp on ACT, row-sums via ones-matmul on PE,
    normalization after PE-transpose back to row-major.
  - ctx redistribution head-shard -> row-shard via per-batch AllToAll.
  - LN + FF + collapse(d->1): row-parallel (core i owns 256 L-positions of
    each batch).
  - c = [B, L] gathered with a tiny AllGather; the final two small matmuls
    (L->L, L->OUT) run redundantly on every core (weights replicated).
Compute dtype: bf16 (f32 accumulation in PSUM); verified ~0.6% rel err.
"""
import sys
import math

for _p in ("/opt/trn_rl_repo", "/opt/trn_rl_repo/concourse"):
    if _p not in sys.path:
        sys.path.insert(0, _p)

import numpy as np
import ml_dtypes

B, L, D, H, OUT = 4, 2048, 1024, 8, 256
DH = D // H          # 128
N = B * L            # 8192 rows
NC = 8               # cores
RPC = N // NC        # 1024 rows per core (as 4 batches x 256 L-positions)
LPC = L // NC        # 256 L-positions per core per batch
EPS = 1e-12

_CACHE = {}


def _build_nc():
    import concourse.bass as bass
    import concourse.tile as tile
    from concourse import bacc, mybir
    from concourse.masks import make_identity

    BF = mybir.dt.bfloat16
    F32 = mybir.dt.float32
    AF = mybir.ActivationFunctionType
    OP = mybir.AluOpType

    nc = bacc.Bacc("TRN2", debug=False, num_devices=NC)

    # ---- parameters (per-core values supplied via in_maps) ----
    xT = nc.dram_tensor("xT", [D, N], BF, kind="ExternalInput")
    xrows = nc.dram_tensor("xrows", [RPC, D], BF, kind="ExternalInput")
    wqkv = nc.dram_tensor("wqkv", [3, D, DH], BF, kind="ExternalInput")
    bqkv = nc.dram_tensor("bqkv", [3, DH], F32, kind="ExternalInput")
    wff = nc.dram_tensor("wff", [D, D], BF, kind="ExternalInput")
    bff = nc.dram_tensor("bff", [D], F32, kind="ExternalInput")
    gamma = nc.dram_tensor("gamma", [D], F32, kind="ExternalInput")
    beta = nc.dram_tensor("beta", [D], F32, kind="ExternalInput")
    wc1 = nc.dram_tensor("wc1", [D, D], BF, kind="ExternalInput")   # gamma-folded
    bc1 = nc.dram_tensor("bc1", [D], F32, kind="ExternalInput")     # beta-folded
    wc2 = nc.dram_tensor("wc2", [D], BF, kind="ExternalInput")
    bc2 = nc.dram_tensor("bc2", [1], F32, kind="ExternalInput")
    wl1 = nc.dram_tensor("wl1", [L, L], BF, kind="ExternalInput")
    bl1 = nc.dram_tensor("bl1", [L], F32, kind="ExternalInput")
    wl2 = nc.dram_tensor("wl2", [L, OUT], BF, kind="ExternalInput")
    bl2 = nc.dram_tensor("bl2", [OUT], F32, kind="ExternalInput")
    out = nc.dram_tensor("out", [B, OUT], F32, kind="ExternalOutput")

    # ---- internal DRAM ----
    a2a_in = nc.dram_tensor("a2a_in", [N, DH], BF)
    a2a_out = nc.dram_tensor("a2a_out", [N, DH], BF)
    sums_hbm = nc.dram_tensor("sums_hbm", [N], F32)
    c_ag_in = nc.dram_tensor("c_ag_in", [RPC], F32)
    c_ag_out = nc.dram_tensor("c_ag_out", [N], F32, addr_space="Shared")

    def bcast(dram_handle, parts, free):
        """Broadcast a [free] DRAM vector across `parts` partitions."""
        ap = dram_handle.ap()
        return bass.AP(tensor=ap.tensor, offset=0, ap=[[0, parts], [1, free]])

    RG = [list(range(NC))]
    ISQ = 1.0  # 1/sqrt(DH) folded into wq/bq on host

    from contextlib import ExitStack

    with tile.TileContext(nc) as tc, ExitStack() as root:
        glob = root.enter_context(tc.tile_pool(name="glob", bufs=1))
        ident = glob.tile([128, 128], BF)
        make_identity(nc, ident[:])
        ones128 = glob.tile([128, 1], BF)
        nc.vector.memset(ones128[:], 1.0)
        eps_sb = glob.tile([128, 1], F32)
        nc.vector.memset(eps_sb[:], EPS)

        phAB = root.enter_context(ExitStack())
        qkv_pool = phAB.enter_context(tc.tile_pool(name="qkv", bufs=1))
        # persistent through phases A+B
        qkvT = qkv_pool.tile([128, 3, N], BF)   # 48KB/part: q^T, k^T, v^T

        # ================= Phase A: QKV^T =================
        with ExitStack() as phA:
            xt_pool = phA.enter_context(tc.tile_pool(name="xt", bufs=1))
            wq_pool = phA.enter_context(tc.tile_pool(name="wqkv", bufs=1))
            psA = phA.enter_context(tc.tile_pool(name="psA", bufs=8, space="PSUM"))

            xt = xt_pool.tile([128, D // 128, N], BF)   # x^T, 128KB/part
            # chunked loads (row-groups of 2048) so the first matmuls start early
            for rg in range(4):
                for kc in range(D // 128):
                    nc.sync.dma_start(
                        out=xt[:, kc, rg * 2048:(rg + 1) * 2048],
                        in_=xT.ap()[kc * 128:(kc + 1) * 128, rg * 2048:(rg + 1) * 2048])
            wq_sb = wq_pool.tile([128, 3, D // 128, DH], BF)
            nc.sync.dma_start(
                out=wq_sb[:],
                in_=bass.AP(tensor=wqkv.ap().tensor, offset=0,
                            ap=[[DH, 128], [D * DH, 3], [128 * DH, D // 128], [1, DH]]))
            bq_sb = wq_pool.tile([128, 3], F32)
            nc.sync.dma_start(
                out=bq_sb[:],
                in_=bass.AP(tensor=bqkv.ap().tensor, offset=0,
                            ap=[[1, 128], [DH, 3]]))

            NRC = N // 512  # 16 chunks of 512 rows
            for s in range(3):
                for g in range(NRC // 4):
                    pst = [psA.tile([128, 512], F32, tag="qkvps", name=f"qkvps{s}_{g}_{j}")
                           for j in range(4)]
                    for kc in range(D // 128):
                        for r4 in range(4):
                            rc = g * 4 + r4
                            nc.tensor.matmul(
                                pst[r4][:], wq_sb[:, s, kc, :],
                                xt[:, kc, rc * 512:(rc + 1) * 512],
                                start=(kc == 0), stop=(kc == D // 128 - 1))
                    for r4 in range(4):
                        rc = g * 4 + r4
                        nc.vector.tensor_scalar_add(
                            qkvT[:, s, rc * 512:(rc + 1) * 512], pst[r4][:],
                            bq_sb[:, s:s + 1])

        # ================= Phase B: attention per batch =================
        with ExitStack() as phB:
            vnat_pool = phB.enter_context(tc.tile_pool(name="vnat", bufs=1))
            pT_pool = phB.enter_context(tc.tile_pool(name="pT", bufs=2))
            ctxT_pool = phB.enter_context(tc.tile_pool(name="ctxT", bufs=2))
            sums_pool = phB.enter_context(tc.tile_pool(name="sums", bufs=1))
            recip_pool = phB.enter_context(tc.tile_pool(name="recip", bufs=2))
            norm_pool = phB.enter_context(tc.tile_pool(name="norm", bufs=3))
            psS = phB.enter_context(tc.tile_pool(name="psS", bufs=2, space="PSUM"))
            psC = phB.enter_context(tc.tile_pool(name="psC", bufs=2, space="PSUM"))
            psSum = phB.enter_context(tc.tile_pool(name="psSum", bufs=1, space="PSUM"))
            psTrB = phB.enter_context(tc.tile_pool(name="psTrB", bufs=1, space="PSUM"))

            sums_sb = sums_pool.tile([1, N], F32)
            KCB = L // 128  # 16 key chunks per batch

            # v row-major via PE transpose
            vnat = vnat_pool.tile([128, N // 128, DH], BF)  # 16KB/part
            for rc in range(N // 128):
                tps = psTrB.tile([128, 128], BF, tag="ctr")
                nc.tensor.transpose(tps[:], qkvT[:, 2, rc * 128:(rc + 1) * 128], ident[:])
                nc.vector.tensor_copy(vnat[:, rc, :], tps[:])

            a2a_insts = []
            for b in range(B):
                ctxT_sb = ctxT_pool.tile([128, L], BF, tag="ctxT")
                for qc in range(L // 1024):
                    pT = pT_pool.tile([128, KCB, 1024], BF, tag="pT")
                    q0 = b * L + qc * 1024
                    for kc in range(KCB):
                        sps = psS.tile([128, 1024], F32, tag="sps")
                        for hh in range(2):
                            nc.tensor.matmul(
                                sps[:, hh * 512:(hh + 1) * 512],
                                qkvT[:, 1, b * L + kc * 128: b * L + (kc + 1) * 128],
                                qkvT[:, 0, q0 + hh * 512: q0 + (hh + 1) * 512],
                                start=True, stop=True)
                        nc.scalar.activation(pT[:, kc, :], sps[:], AF.Exp, scale=ISQ)
                    for hh in range(2):
                        cps = psC.tile([128, 512], F32, tag="cps")
                        sps2 = psSum.tile([1, 512], F32, tag="sps2")
                        for kc in range(KCB):
                            nc.tensor.matmul(cps[:], vnat[:, b * KCB + kc, :],
                                             pT[:, kc, hh * 512:(hh + 1) * 512],
                                             start=(kc == 0), stop=(kc == KCB - 1))
                        for kc in range(KCB):
                            nc.tensor.matmul(sps2[:], ones128[:],
                                             pT[:, kc, hh * 512:(hh + 1) * 512],
                                             start=(kc == 0), stop=(kc == KCB - 1))
                        nc.vector.tensor_copy(
                            ctxT_sb[:, qc * 1024 + hh * 512: qc * 1024 + (hh + 1) * 512],
                            cps[:])
                        nc.vector.tensor_copy(
                            sums_sb[:, q0 + hh * 512: q0 + (hh + 1) * 512], sps2[:])
                # reciprocal of sums, transposed into per-row columns
                nc.sync.dma_start(out=sums_hbm.ap()[b * L:(b + 1) * L],
                                  in_=sums_sb[0:1, b * L:(b + 1) * L])
                rraw = recip_pool.tile([128, KCB], F32, tag="rraw")
                nc.sync.dma_start(
                    out=rraw[:],
                    in_=sums_hbm.ap()[b * L:(b + 1) * L].rearrange("(j p) -> p j", p=128))
                rcols = recip_pool.tile([128, KCB], F32, tag="rcols")
                nc.vector.reciprocal(rcols[:], rraw[:])
                # transpose ctx^T back to row-major, normalize, store to a2a_in
                for rc in range(KCB):
                    tps = psTrB.tile([128, 128], BF, tag="ctr")
                    nc.tensor.transpose(tps[:], ctxT_sb[:, rc * 128:(rc + 1) * 128], ident[:])
                    nrm = norm_pool.tile([128, DH], BF, tag="nrm")
                    nc.vector.tensor_scalar_mul(nrm[:], tps[:], rcols[:, rc:rc + 1])
                    nc.sync.dma_start(
                        out=a2a_in.ap()[b * L + rc * 128: b * L + (rc + 1) * 128, :],
                        in_=nrm[:])
                a2a_insts.append(nc.gpsimd.collective_compute(
                    "AllToAll", OP.bypass,
                    ins=[a2a_in.ap()[b * L:(b + 1) * L, :]],
                    outs=[a2a_out.ap()[b * L:(b + 1) * L, :]],
                    replica_groups=RG))
        phAB.close()  # release qkvT before Phase C

        # ================= Phase C: row-parallel LN/FF/collapse =================
        with ExitStack() as phC:
            wC_pool = phC.enter_context(tc.tile_pool(name="wC", bufs=1))
            rowC = phC.enter_context(tc.tile_pool(name="rowC", bufs=2))
            h2T_pool = phC.enter_context(tc.tile_pool(name="h2T", bufs=1))
            psFF = phC.enter_context(tc.tile_pool(name="psFF", bufs=2, space="PSUM"))
            psTrC = phC.enter_context(tc.tile_pool(name="psTrC", bufs=2, space="PSUM"))
            psC1 = phC.enter_context(tc.tile_pool(name="psC1", bufs=2, space="PSUM"))
            psC2 = phC.enter_context(tc.tile_pool(name="psC2", bufs=1, space="PSUM"))
            psFin = phC.enter_context(tc.tile_pool(name="psFin", bufs=1, space="PSUM"))

            from concourse.tile_rust import add_dep_helper as _adh

            def _delay(dma_inst):
                # keep big phase-C weight loads off the DMA queues until the
                # phase-A/B input traffic is done
                _adh(dma_inst.ins, a2a_insts[0].ins, sync=True,
                     reason="defer phase-C weight load")
                return dma_inst

            DKC = D // 128  # 8
            wff_sb = wC_pool.tile([128, DKC, D], BF)
            _delay(nc.sync.dma_start(
                out=wff_sb[:],
                in_=bass.AP(tensor=wff.ap().tensor, offset=0,
                            ap=[[D, 128], [128 * D, DKC], [1, D]])))
            wc1_sb = wC_pool.tile([128, DKC, D], BF)
            _delay(nc.sync.dma_start(
                out=wc1_sb[:],
                in_=bass.AP(tensor=wc1.ap().tensor, offset=0,
                            ap=[[D, 128], [128 * D, DKC], [1, D]])))
            wc2_sb = wC_pool.tile([128, DKC], BF)
            nc.sync.dma_start(
                out=wc2_sb[:],
                in_=bass.AP(tensor=wc2.ap().tensor, offset=0,
                            ap=[[1, 128], [128, DKC]]))
            bc1_sb = wC_pool.tile([128, DKC], F32)
            nc.sync.dma_start(
                out=bc1_sb[:],
                in_=bass.AP(tensor=bc1.ap().tensor, offset=0,
                            ap=[[1, 128], [128, DKC]]))
            bc2_sb = wC_pool.tile([1, 1], F32)
            nc.sync.dma_start(out=bc2_sb[:], in_=bc2.ap())
            gamma_bc = wC_pool.tile([128, D], F32)
            nc.sync.dma_start(out=gamma_bc[:], in_=bcast(gamma, 128, D))
            beta_bc = wC_pool.tile([128, D], F32)
            nc.sync.dma_start(out=beta_bc[:], in_=bcast(beta, 128, D))
            bff_bc = wC_pool.tile([128, D], F32)
            nc.sync.dma_start(out=bff_bc[:], in_=bcast(bff, 128, D))
            # large final weights (wl1 loaded on demand per 512-col chunk)
            wl1_pool = phC.enter_context(tc.tile_pool(name="wl1", bufs=2))
            wl2_sb = wC_pool.tile([128, L // 128, OUT], BF)
            _delay(nc.sync.dma_start(
                out=wl2_sb[:],
                in_=bass.AP(tensor=wl2.ap().tensor, offset=0,
                            ap=[[OUT, 128], [128 * OUT, L // 128], [1, OUT]])))

            h2T_all = h2T_pool.tile([128, DKC, RPC], BF)
            c2_sb = h2T_pool.tile([1, RPC], F32)

            def layernorm_rows(src_f32, dst, apply_gb):
                """src [128, D] f32 -> dst (normalized; optionally *gamma+beta)."""
                stats = rowC.tile([128, 2, nc.vector.BN_STATS_DIM], F32, tag="stats")
                for sg in range(2):
                    nc.vector.bn_stats(stats[:, sg, :], src_f32[:, sg * 512:(sg + 1) * 512])
                mv = rowC.tile([128, nc.vector.BN_AGGR_DIM], F32, tag="mv")
                nc.vector.bn_aggr(mv[:], stats[:])
                sq = rowC.tile([128, 1], F32, tag="sq")
                nc.scalar.activation(sq[:], mv[:, 1:2], AF.Sqrt, bias=eps_sb[:], scale=1.0)
                rstd = rowC.tile([128, 1], F32, tag="rstd")
                nc.vector.reciprocal(rstd[:], sq[:])
                if apply_gb:
                    z = rowC.tile([128, D], F32, tag="zf")
                    nc.vector.tensor_scalar(z[:], src_f32[:], mv[:, 0:1], rstd[:],
                                            op0=OP.subtract, op1=OP.mult)
                    zg = rowC.tile([128, D], F32, tag="zg")
                    nc.vector.tensor_mul(zg[:], z[:], gamma_bc[:])
                    nc.vector.tensor_add(dst[:], zg[:], beta_bc[:])
                else:
                    nc.vector.tensor_scalar(dst[:], src_f32[:], mv[:, 0:1], rstd[:],
                                            op0=OP.subtract, op1=OP.mult)

            for t in range(RPC // 128):
                b, e = t // 2, t % 2
                # rows of this tile: batch b, L-positions [i*LPC + e*128 + p)
                # (core-specific x rows arrive pre-sliced via `xrows`)
                ctx_t = rowC.tile([128, H, DH], BF, tag="ctx_t")
                nc.sync.dma_start(
                    out=ctx_t[:],
                    in_=bass.AP(tensor=a2a_out.ap().tensor,
                                offset=(b * L + e * 128) * DH,
                                ap=[[DH, 128], [LPC * DH, H], [1, DH]]))
                x_t = rowC.tile([128, D], BF, tag="x_t")
                nc.sync.dma_start(out=x_t[:], in_=xrows.ap()[t * 128:(t + 1) * 128, :])
                s_t = rowC.tile([128, D], F32, tag="s_t")
                nc.vector.tensor_add(s_t[:], x_t[:], ctx_t[:].rearrange("p h d -> p (h d)"))
                h1b = rowC.tile([128, D], BF, tag="h1b")
                layernorm_rows(s_t, h1b, apply_gb=True)
                # h1^T for the ff matmul
                h1T = rowC.tile([128, DKC, 128], BF, tag="h1T")
                for kc in range(DKC):
                    tps = psTrC.tile([128, 128], BF, tag="htr")
                    nc.tensor.transpose(tps[:], h1b[:, kc * 128:(kc + 1) * 128], ident[:])
                    nc.vector.tensor_copy(h1T[:, kc, :], tps[:])
                # ff natural [128 rows, D]
                f_t = rowC.tile([128, D], BF, tag="f_t")
                for dc in range(2):
                    fps = psFF.tile([128, 512], F32, tag="fps")
                    for kc in range(DKC):
                        nc.tensor.matmul(fps[:], h1T[:, kc, :],
                                         wff_sb[:, kc, dc * 512:(dc + 1) * 512],
                                         start=(kc == 0), stop=(kc == DKC - 1))
                    tmp = rowC.tile([128, 512], F32, tag="fftmp")
                    nc.vector.tensor_add(tmp[:], fps[:], bff_bc[:, dc * 512:(dc + 1) * 512])
                    nc.vector.tensor_scalar_max(f_t[:, dc * 512:(dc + 1) * 512], tmp[:], 0.0)
                s2_t = rowC.tile([128, D], F32, tag="s2_t")
                nc.vector.tensor_add(s2_t[:], h1b[:], f_t[:])
                h2b = rowC.tile([128, D], BF, tag="h2b")
                layernorm_rows(s2_t, h2b, apply_gb=False)  # gamma/beta folded into wc1
                for kc in range(DKC):
                    tps = psTrC.tile([128, 128], BF, tag="htr")
                    nc.tensor.transpose(tps[:], h2b[:, kc * 128:(kc + 1) * 128], ident[:])
                    nc.vector.tensor_copy(h2T_all[:, kc, t * 128:(t + 1) * 128], tps[:])

            # c1^T = relu(wc1'^T h2 + bc1') ; then c2 = relu(c1 @ wc2 + bc2)
            c1T = h2T_pool.tile([128, DKC, RPC], BF)
            for fc in range(DKC):
                for rc in range(RPC // 512):
                    cps = psC1.tile([128, 512], F32, tag="c1ps")
                    for kc in range(DKC):
                        nc.tensor.matmul(cps[:], wc1_sb[:, kc, fc * 128:(fc + 1) * 128],
                                         h2T_all[:, kc, rc * 512:(rc + 1) * 512],
                                         start=(kc == 0), stop=(kc == DKC - 1))
                    nc.scalar.activation(c1T[:, fc, rc * 512:(rc + 1) * 512], cps[:],
                                         AF.Relu, bias=bc1_sb[:, fc:fc + 1], scale=1.0)
            for rc in range(RPC // 512):
                c2ps = psC2.tile([1, 512], F32, tag="c2ps")
                for kc in range(DKC):
                    nc.tensor.matmul(c2ps[:], wc2_sb[:, kc:kc + 1],
                                     c1T[:, kc, rc * 512:(rc + 1) * 512],
                                     start=(kc == 0), stop=(kc == DKC - 1))
                nc.scalar.activation(c2_sb[0:1, rc * 512:(rc + 1) * 512], c2ps[:],
                                     AF.Relu, bias=bc2_sb[0:1, :], scale=1.0)
            nc.sync.dma_start(out=c_ag_in.ap().rearrange("(o n) -> o n", o=1),
                              in_=c2_sb[0:1, :])
            nc.gpsimd.collective_compute(
                "AllGather", OP.bypass,
                ins=[c_ag_in.ap()], outs=[c_ag_out.ap()], replica_groups=RG)

            # final: out = relu(c @ wl1 + bl1) @ wl2 + bl2 (redundant on all cores)
            cT = rowC.tile([128, 2, NC, B], F32, tag="cT")
            for e in range(2):
                nc.sync.dma_start(
                    out=cT[:, e, :, :],
                    in_=bass.AP(tensor=c_ag_out.ap().tensor, offset=e * 128,
                                ap=[[1, 128], [RPC, NC], [LPC, B]]))
            cTb = rowC.tile([128, 2, NC, B], BF, tag="cTb")
            nc.vector.tensor_copy(cTb[:], cT[:])
            bl1_bc = wC_pool.tile([B, L], F32)
            nc.sync.dma_start(out=bl1_bc[:], in_=bcast(bl1, B, L))
            bl2_bc = wC_pool.tile([B, OUT], F32)
            nc.sync.dma_start(out=bl2_bc[:], in_=bcast(bl2, B, OUT))
            c1f = rowC.tile([B, L], BF, tag="c1f")
            for oc in range(L // 512):
                wl1_oc = wl1_pool.tile([128, L // 128, 512], BF, tag="wl1oc")
                _delay(nc.sync.dma_start(
                    out=wl1_oc[:],
                    in_=bass.AP(tensor=wl1.ap().tensor, offset=oc * 512,
                                ap=[[L, 128], [128 * L, L // 128], [1, 512]])))
                fps = psFin.tile([B, 512], F32, tag="finps")
                for kc in range(L // 128):
                    nc.tensor.matmul(fps[:], cTb[:, kc % 2, kc // 2, :],
                                     wl1_oc[:, kc, :],
                                     start=(kc == 0), stop=(kc == L // 128 - 1))
                tmp = rowC.tile([B, 512], F32, tag="fintmp")
                nc.vector.tensor_add(tmp[:], fps[:], bl1_bc[:, oc * 512:(oc + 1) * 512])
                nc.vector.tensor_scalar_max(c1f[:, oc * 512:(oc + 1) * 512], tmp[:], 0.0)
            c1fT = rowC.tile([128, L // 128, B], BF, tag="c1fT")
            for j in range(L // 128):
                tps = psTrC.tile([128, B], BF, tag="htr")
                nc.tensor.transpose(tps[:], c1f[0:B, j * 128:(j + 1) * 128], ident[0:B, 0:B])
                nc.vector.tensor_copy(c1fT[:, j, :], tps[:])
            ops = psFin.tile([B, OUT], F32, tag="finps")
            for kc in range(L // 128):
                nc.tensor.matmul(ops[:], c1fT[:, kc, :], wl2_sb[:, kc, :],
                                 start=(kc == 0), stop=(kc == L // 128 - 1))
            out_f = rowC.tile([B, OUT], F32, tag="out_f")
            nc.vector.tensor_add(out_f[:], ops[:], bl2_bc[:])
            nc.sync.dma_start(out=out.ap(), in_=out_f[:])

    nc.compile()
    return nc


def _to_bf16(a):
    return np.asarray(a, dtype=np.float32).astype(ml_dtypes.bfloat16)


def kernel(**inputs):
    from concourse.bass_utils import run_bass_kernel_spmd

    if "nc" not in _CACHE:
        _CACHE["nc"] = _build_nc()
    nc = _CACHE["nc"]

    x = np.asarray(inputs["x"], dtype=np.float32).reshape(N, D)
    isq = 1.0 / math.sqrt(DH)
    gamma_np = np.asarray(inputs["gamma"], dtype=np.float32)
    beta_np = np.asarray(inputs["beta"], dtype=np.float32)
    wc1_np = np.asarray(inputs["wc1"], dtype=np.float32)
    bc1_np = np.asarray(inputs["bc1"], dtype=np.float32)
    # fold LN2's gamma/beta into the c1 projection (h2 feeds only this matmul)
    wc1_f = gamma_np[:, None] * wc1_np
    bc1_f = bc1_np + beta_np @ wc1_np

    xT_bf = np.ascontiguousarray(_to_bf16(x).T)
    shared = dict(
        xT=xT_bf,
        wff=_to_bf16(inputs["wff"]),
        bff=np.asarray(inputs["bff"], np.float32),
        gamma=gamma_np, beta=beta_np,
        wc1=_to_bf16(wc1_f), bc1=bc1_f.astype(np.float32),
        wc2=_to_bf16(np.asarray(inputs["wc2"]).reshape(D)),
        bc2=np.asarray(inputs["bc2"], np.float32).reshape(1),
        wl1=_to_bf16(inputs["wl1"]), bl1=np.asarray(inputs["bl1"], np.float32),
        wl2=_to_bf16(inputs["wl2"]), bl2=np.asarray(inputs["bl2"], np.float32),
    )
    wq = np.asarray(inputs["wq"], np.float32) * isq
    bq = np.asarray(inputs["bq"], np.float32) * isq
    wk = np.asarray(inputs["wk"], np.float32)
    bk = np.asarray(inputs["bk"], np.float32)
    wv = np.asarray(inputs["wv"], np.float32)
    bv = np.asarray(inputs["bv"], np.float32)

    in_maps = []
    for i in range(NC):
        sl = slice(i * DH, (i + 1) * DH)
        wqkv_i = np.stack([wq[:, sl], wk[:, sl], wv[:, sl]])
        bqkv_i = np.stack([bq[sl], bk[sl], bv[sl]])
        # rows this core owns after the A2A: for each batch b, L-positions
        # [i*LPC, (i+1)*LPC) -> 8 row-tiles of 128 = (b, e) pairs
        xr = np.concatenate([
            x[b * L + i * LPC: b * L + (i + 1) * LPC, :] for b in range(B)
        ])  # [RPC, D] ordered (b, l-within-block)
        in_maps.append(dict(
            shared,
            wqkv=_to_bf16(wqkv_i),
            bqkv=bqkv_i.astype(np.float32),
            xrows=_to_bf16(xr),
        ))

    res = run_bass_kernel_spmd(nc, in_maps, core_ids=list(range(NC)))
    return np.asarray(res.results[0]["out"], dtype=np.float32)


# revision 34
# speedup vs baseline: 1.1853x; 1.0727x over previous
"""Distributed Trainium2 kernel for nn_Attention_64854006169830.

Strategy (8 NeuronCores, SPMD):
  - QKV + attention: head-parallel (core i computes head i for all B*L rows),
    with activations kept feature-major ("transposed") so every matmul uses
    natural weight layouts. Softmax is computed on transposed scores
    (keys on partitions): exp on ACT, row-sums via ones-matmul on PE,
    normalization after PE-transpose back to row-major.
  - ctx redistribution head-shard -> row-shard via per-batch AllToAll.
  - LN + FF + collapse(d->1): row-parallel (core i owns 256 L-positions of
    each batch).
  - c = [B, L] gathered with a tiny AllGather; the final two small matmuls
    (L->L, L->OUT) run redundantly on every core (weights replicated).
Compute dtype: bf16 (f32 accumulation in PSUM); verified ~0.6% rel err.
"""
import sys
import math

for _p in ("/opt/trn_rl_repo", "/opt/trn_rl_repo/concourse"):
    if _p not in sys.path:
        sys.path.insert(0, _p)

import numpy as np
import ml_dtypes

B, L, D, H, OUT = 4, 2048, 1024, 8, 256
DH = D // H          # 128
N = B * L            # 8192 rows
NC = 8               # cores
RPC = N // NC        # 1024 rows per core (as 4 batches x 256 L-positions)
LPC = L // NC        # 256 L-positions per core per batch
EPS = 1e-12

_CACHE = {}


def _build_nc():
    import concourse.bass as bass
    import concourse.tile as tile
    from concourse import bacc, mybir
    from concourse.masks import make_identity

    BF = mybir.dt.bfloat16
    F32 = mybir.dt.float32
    AF = mybir.ActivationFunctionType
    OP = mybir.AluOpType

    nc = bacc.Bacc("TRN2", debug=False, num_devices=NC)

    # ---- parameters (per-core values supplied via in_maps) ----
    xT = nc.dram_tensor("xT", [D, N], BF, kind="ExternalInput")
    xrows = nc.dram_tensor("xrows", [RPC, D], BF, kind="ExternalInput")
    wqkv = nc.dram_tensor("wqkv", [3, D, DH], BF, kind="ExternalInput")
    bqkv = nc.dram_tensor("bqkv", [3, DH], F32, kind="ExternalInput")
    wff = nc.dram_tensor("wff", [D, D], BF, kind="ExternalInput")
    bff = nc.dram_tensor("bff", [D], F32, kind="ExternalInput")
    gamma = nc.dram_tensor("gamma", [D], BF, kind="ExternalInput")
    beta = nc.dram_tensor("beta", [D], BF, kind="ExternalInput")
    wc1 = nc.dram_tensor("wc1", [D, D], BF, kind="ExternalInput")   # gamma-folded
    bc1 = nc.dram_tensor("bc1", [D], F32, kind="ExternalInput")     # beta-folded
    wc2 = nc.dram_tensor("wc2", [D], BF, kind="ExternalInput")
    bc2 = nc.dram_tensor("bc2", [1], F32, kind="ExternalInput")
    wl1 = nc.dram_tensor("wl1", [L, L], BF, kind="ExternalInput")
    bl1 = nc.dram_tensor("bl1", [L], F32, kind="ExternalInput")
    wl2 = nc.dram_tensor("wl2", [L, OUT], BF, kind="ExternalInput")
    bl2 = nc.dram_tensor("bl2", [OUT], F32, kind="ExternalInput")
    out = nc.dram_tensor("out", [B, OUT], F32, kind="ExternalOutput")

    # ---- internal DRAM ----
    a2a_in = nc.dram_tensor("a2a_in", [N, DH], BF)
    a2a_out = nc.dram_tensor("a2a_out", [N, DH], BF)
    sums_hbm = nc.dram_tensor("sums_hbm", [N], F32)
    c_ag_in = nc.dram_tensor("c_ag_in", [RPC], F32)
    c_ag_out = nc.dram_tensor("c_ag_out", [N], F32, addr_space="Shared")

    def bcast(dram_handle, parts, free):
        """Broadcast a [free] DRAM vector across `parts` partitions."""
        ap = dram_handle.ap()
        return bass.AP(tensor=ap.tensor, offset=0, ap=[[0, parts], [1, free]])

    RG = [list(range(NC))]
    ISQ = 1.0  # 1/sqrt(DH) folded into wq/bq on host

    from contextlib import ExitStack

    with tile.TileContext(nc) as tc, ExitStack() as root:
        glob = root.enter_context(tc.tile_pool(name="glob", bufs=1))
        ident = glob.tile([128, 128], BF)
        make_identity(nc, ident[:])
        ones128 = glob.tile([128, 1], BF)
        nc.vector.memset(ones128[:], 1.0)
        eps_sb = glob.tile([128, 1], F32)
        nc.vector.memset(eps_sb[:], EPS)

        phAB = root.enter_context(ExitStack())
        qkv_pool = phAB.enter_context(tc.tile_pool(name="qkv", bufs=1))
        # persistent through phases A+B
        qkvT = qkv_pool.tile([128, 3, N], BF)   # 48KB/part: q^T, k^T, v^T

        # ================= Phase A: QKV^T =================
        with ExitStack() as phA:
            xt_pool = phA.enter_context(tc.tile_pool(name="xt", bufs=1))
            wq_pool = phA.enter_context(tc.tile_pool(name="wqkv", bufs=1))
            psA = phA.enter_context(tc.tile_pool(name="psA", bufs=8, space="PSUM"))

            # weights first (tiny) so the first matmuls aren't stuck behind
            # the 16MB x^T load in the DMA queues
            wq_sb = wq_pool.tile([128, 3, D // 128, DH], BF)
            nc.sync.dma_start(
                out=wq_sb[:],
                in_=bass.AP(tensor=wqkv.ap().tensor, offset=0,
                            ap=[[DH, 128], [D * DH, 3], [128 * DH, D // 128], [1, DH]]))
            bq_sb = wq_pool.tile([128, 3], F32)
            nc.sync.dma_start(
                out=bq_sb[:],
                in_=bass.AP(tensor=bqkv.ap().tensor, offset=0,
                            ap=[[1, 128], [DH, 3]]))

            xt = xt_pool.tile([128, D // 128, N], BF)   # x^T, 128KB/part
            # chunked loads (row-groups of 2048) so the first matmuls start early
            for rg in range(4):
                for kc in range(D // 128):
                    nc.sync.dma_start(
                        out=xt[:, kc, rg * 2048:(rg + 1) * 2048],
                        in_=xT.ap()[kc * 128:(kc + 1) * 128, rg * 2048:(rg + 1) * 2048])

            # row-group-major so compute on group g starts right after its DMA
            for rg in range(4):
                for s in range(3):
                    pst = [psA.tile([128, 512], F32, tag="qkvps", name=f"qkvps{rg}_{s}_{j}")
                           for j in range(4)]
                    for kc in range(D // 128):
                        for r4 in range(4):
                            rc = rg * 4 + r4
                            nc.tensor.matmul(
                                pst[r4][:], wq_sb[:, s, kc, :],
                                xt[:, kc, rc * 512:(rc + 1) * 512],
                                start=(kc == 0), stop=(kc == D // 128 - 1))
                    for r4 in range(4):
                        rc = rg * 4 + r4
                        nc.vector.tensor_scalar_add(
                            qkvT[:, s, rc * 512:(rc + 1) * 512], pst[r4][:],
                            bq_sb[:, s:s + 1])

        # ================= Phase B: attention per batch =================
        with ExitStack() as phB:
            vnat_pool = phB.enter_context(tc.tile_pool(name="vnat", bufs=1))
            pT_pool = phB.enter_context(tc.tile_pool(name="pT", bufs=2))
            ctxT_pool = phB.enter_context(tc.tile_pool(name="ctxT", bufs=2))
            sums_pool = phB.enter_context(tc.tile_pool(name="sums", bufs=1))
            recip_pool = phB.enter_context(tc.tile_pool(name="recip", bufs=2))
            norm_pool = phB.enter_context(tc.tile_pool(name="norm", bufs=3))
            psS = phB.enter_context(tc.tile_pool(name="psS", bufs=2, space="PSUM"))
            psC = phB.enter_context(tc.tile_pool(name="psC", bufs=2, space="PSUM"))
            psSum = phB.enter_context(tc.tile_pool(name="psSum", bufs=1, space="PSUM"))
            psTrB = phB.enter_context(tc.tile_pool(name="psTrB", bufs=1, space="PSUM"))

            sums_sb = sums_pool.tile([1, N], F32)
            KCB = L // 128  # 16 key chunks per batch

            # v row-major via PE transpose
            vnat = vnat_pool.tile([128, N // 128, DH], BF)  # 16KB/part
            for rc in range(N // 128):
                tps = psTrB.tile([128, 128], BF, tag="ctr")
                nc.tensor.transpose(tps[:], qkvT[:, 2, rc * 128:(rc + 1) * 128], ident[:])
                nc.vector.tensor_copy(vnat[:, rc, :], tps[:])

            a2a_insts = []
            for b in range(B):
                ctxT_sb = ctxT_pool.tile([128, L], BF, tag="ctxT")
                for qc in range(L // 1024):
                    pT = pT_pool.tile([128, KCB, 1024], BF, tag="pT")
                    q0 = b * L + qc * 1024
                    for kc in range(KCB):
                        sps = psS.tile([128, 1024], F32, tag="sps")
                        for hh in range(2):
                            nc.tensor.matmul(
                                sps[:, hh * 512:(hh + 1) * 512],
                                qkvT[:, 1, b * L + kc * 128: b * L + (kc + 1) * 128],
                                qkvT[:, 0, q0 + hh * 512: q0 + (hh + 1) * 512],
                                start=True, stop=True)
                        nc.scalar.activation(pT[:, kc, :], sps[:], AF.Exp, scale=ISQ)
                    for hh in range(2):
                        cps = psC.tile([128, 512], F32, tag="cps")
                        sps2 = psSum.tile([1, 512], F32, tag="sps2")
                        for kc in range(KCB):
                            nc.tensor.matmul(cps[:], vnat[:, b * KCB + kc, :],
                                             pT[:, kc, hh * 512:(hh + 1) * 512],
                                             start=(kc == 0), stop=(kc == KCB - 1))
                        for kc in range(KCB):
                            nc.tensor.matmul(sps2[:], ones128[:],
                                             pT[:, kc, hh * 512:(hh + 1) * 512],
                                             start=(kc == 0), stop=(kc == KCB - 1))
                        nc.vector.tensor_copy(
                            ctxT_sb[:, qc * 1024 + hh * 512: qc * 1024 + (hh + 1) * 512],
                            cps[:])
                        nc.vector.tensor_copy(
                            sums_sb[:, q0 + hh * 512: q0 + (hh + 1) * 512], sps2[:])
                # reciprocal of sums, transposed into per-row columns
                nc.sync.dma_start(out=sums_hbm.ap()[b * L:(b + 1) * L],
                                  in_=sums_sb[0:1, b * L:(b + 1) * L])
                rraw = recip_pool.tile([128, KCB], F32, tag="rraw")
                nc.sync.dma_start(
                    out=rraw[:],
                    in_=sums_hbm.ap()[b * L:(b + 1) * L].rearrange("(j p) -> p j", p=128))
                rcols = recip_pool.tile([128, KCB], F32, tag="rcols")
                nc.vector.reciprocal(rcols[:], rraw[:])
                # transpose ctx^T back to row-major, normalize, store to a2a_in
                for rc in range(KCB):
                    tps = psTrB.tile([128, 128], BF, tag="ctr")
                    nc.tensor.transpose(tps[:], ctxT_sb[:, rc * 128:(rc + 1) * 128], ident[:])
                    nrm = norm_pool.tile([128, DH], BF, tag="nrm")
                    nc.vector.tensor_scalar_mul(nrm[:], tps[:], rcols[:, rc:rc + 1])
                    nc.sync.dma_start(
                        out=a2a_in.ap()[b * L + rc * 128: b * L + (rc + 1) * 128, :],
                        in_=nrm[:])
                a2a_insts.append(nc.gpsimd.collective_compute(
                    "AllToAll", OP.bypass,
                    ins=[a2a_in.ap()[b * L:(b + 1) * L, :]],
                    outs=[a2a_out.ap()[b * L:(b + 1) * L, :]],
                    replica_groups=RG))
        phAB.close()  # release qkvT before Phase C

        # ================= Phase C: row-parallel LN/FF/collapse =================
        with ExitStack() as phC:
            wC_pool = phC.enter_context(tc.tile_pool(name="wC", bufs=1))
            rowC = phC.enter_context(tc.tile_pool(name="rowC", bufs=2))
            h2T_pool = phC.enter_context(tc.tile_pool(name="h2T", bufs=1))
            psFF = phC.enter_context(tc.tile_pool(name="psFF", bufs=2, space="PSUM"))
            psTrC = phC.enter_context(tc.tile_pool(name="psTrC", bufs=2, space="PSUM"))
            psC1 = phC.enter_context(tc.tile_pool(name="psC1", bufs=2, space="PSUM"))
            psC2 = phC.enter_context(tc.tile_pool(name="psC2", bufs=1, space="PSUM"))
            psFin = phC.enter_context(tc.tile_pool(name="psFin", bufs=1, space="PSUM"))

            from concourse.tile_rust import add_dep_helper as _adh

            def _delay(dma_inst):
                # keep big phase-C weight loads off the DMA queues until the
                # phase-A/B input traffic is done
                _adh(dma_inst.ins, a2a_insts[0].ins, sync=True,
                     reason="defer phase-C weight load")
                return dma_inst

            DKC = D // 128  # 8
            wff_sb = wC_pool.tile([128, DKC, D], BF)
            _delay(nc.sync.dma_start(
                out=wff_sb[:],
                in_=bass.AP(tensor=wff.ap().tensor, offset=0,
                            ap=[[D, 128], [128 * D, DKC], [1, D]])))
            wc1_sb = wC_pool.tile([128, DKC, D], BF)
            _delay(nc.sync.dma_start(
                out=wc1_sb[:],
                in_=bass.AP(tensor=wc1.ap().tensor, offset=0,
                            ap=[[D, 128], [128 * D, DKC], [1, D]])))
            wc2_sb = wC_pool.tile([128, DKC], BF)
            nc.sync.dma_start(
                out=wc2_sb[:],
                in_=bass.AP(tensor=wc2.ap().tensor, offset=0,
                            ap=[[1, 128], [128, DKC]]))
            bc1_sb = wC_pool.tile([128, DKC], F32)
            nc.sync.dma_start(
                out=bc1_sb[:],
                in_=bass.AP(tensor=bc1.ap().tensor, offset=0,
                            ap=[[1, 128], [128, DKC]]))
            bc2_sb = wC_pool.tile([1, 1], F32)
            nc.sync.dma_start(out=bc2_sb[:], in_=bc2.ap())
            gamma_bc = wC_pool.tile([128, D], BF)
            nc.sync.dma_start(out=gamma_bc[:], in_=bcast(gamma, 128, D))
            beta_bc = wC_pool.tile([128, D], BF)
            nc.sync.dma_start(out=beta_bc[:], in_=bcast(beta, 128, D))
            bff_bc = wC_pool.tile([128, D], F32)
            nc.sync.dma_start(out=bff_bc[:], in_=bcast(bff, 128, D))
            # large final weights (wl1 loaded on demand per 512-col chunk)
            wl1_pool = phC.enter_context(tc.tile_pool(name="wl1", bufs=2))
            wl2_sb = wC_pool.tile([128, L // 128, OUT], BF)
            _delay(nc.sync.dma_start(
                out=wl2_sb[:],
                in_=bass.AP(tensor=wl2.ap().tensor, offset=0,
                            ap=[[OUT, 128], [128 * OUT, L // 128], [1, OUT]])))

            h2T_all = h2T_pool.tile([128, DKC, RPC], BF)
            c2_sb = h2T_pool.tile([1, RPC], F32)

            def layernorm_rows(src, dst, apply_gb):
                """src [128, D] bf16 -> dst bf16 (normalized; opt. *gamma+beta)."""
                stats = rowC.tile([128, 2, nc.vector.BN_STATS_DIM], F32, tag="stats")
                for sg in range(2):
                    nc.vector.bn_stats(stats[:, sg, :], src[:, sg * 512:(sg + 1) * 512])
                mv = rowC.tile([128, nc.vector.BN_AGGR_DIM], F32, tag="mv")
                nc.vector.bn_aggr(mv[:], stats[:])
                sq = rowC.tile([128, 1], F32, tag="sq")
                nc.scalar.activation(sq[:], mv[:, 1:2], AF.Sqrt, bias=eps_sb[:], scale=1.0)
                rstd = rowC.tile([128, 1], F32, tag="rstd")
                nc.vector.reciprocal(rstd[:], sq[:])
                if apply_gb:
                    z = rowC.tile([128, D], BF, tag="zf")
                    nc.vector.tensor_scalar(z[:], src[:], mv[:, 0:1], rstd[:],
                                            op0=OP.subtract, op1=OP.mult)
                    zg = rowC.tile([128, D], BF, tag="zg")
                    nc.vector.tensor_mul(zg[:], z[:], gamma_bc[:])
                    nc.vector.tensor_add(dst[:], zg[:], beta_bc[:])
                else:
                    nc.vector.tensor_scalar(dst[:], src[:], mv[:, 0:1], rstd[:],
                                            op0=OP.subtract, op1=OP.mult)

            for t in range(RPC // 128):
                b, e = t // 2, t % 2
                # rows of this tile: batch b, L-positions [i*LPC + e*128 + p)
                # (core-specific x rows arrive pre-sliced via `xrows`)
                ctx_t = rowC.tile([128, H, DH], BF, tag="ctx_t")
                nc.sync.dma_start(
                    out=ctx_t[:],
                    in_=bass.AP(tensor=a2a_out.ap().tensor,
                                offset=(b * L + e * 128) * DH,
                                ap=[[DH, 128], [LPC * DH, H], [1, DH]]))
                x_t = rowC.tile([128, D], BF, tag="x_t")
                nc.sync.dma_start(out=x_t[:], in_=xrows.ap()[t * 128:(t + 1) * 128, :])
                s_t = rowC.tile([128, D], BF, tag="s_t")
                nc.gpsimd.tensor_add(s_t[:], x_t[:], ctx_t[:].rearrange("p h d -> p (h d)"))
                h1b = rowC.tile([128, D], BF, tag="h1b")
                layernorm_rows(s_t, h1b, apply_gb=True)
                # h1^T for the ff matmul
                h1T = rowC.tile([128, DKC, 128], BF, tag="h1T")
                for kc in range(DKC):
                    tps = psTrC.tile([128, 128], BF, tag="htr")
                    nc.tensor.transpose(tps[:], h1b[:, kc * 128:(kc + 1) * 128], ident[:])
                    nc.vector.tensor_copy(h1T[:, kc, :], tps[:])
                # ff natural [128 rows, D]
                f_t = rowC.tile([128, D], BF, tag="f_t")
                for dc in range(2):
                    fps = psFF.tile([128, 512], F32, tag="fps")
                    for kc in range(DKC):
                        nc.tensor.matmul(fps[:], h1T[:, kc, :],
                                         wff_sb[:, kc, dc * 512:(dc + 1) * 512],
                                         start=(kc == 0), stop=(kc == DKC - 1))
                    tmp = rowC.tile([128, 512], BF, tag="fftmp")
                    nc.vector.tensor_add(tmp[:], fps[:], bff_bc[:, dc * 512:(dc + 1) * 512])
                    nc.scalar.activation(f_t[:, dc * 512:(dc + 1) * 512], tmp[:],
                                         AF.Relu, bias=0.0, scale=1.0)
                s2_t = rowC.tile([128, D], BF, tag="s2_t")
                nc.gpsimd.tensor_add(s2_t[:], h1b[:], f_t[:])
                h2b = rowC.tile([128, D], BF, tag="h2b")
                layernorm_rows(s2_t, h2b, apply_gb=False)  # gamma/beta folded into wc1
                for kc in range(DKC):
                    tps = psTrC.tile([128, 128], BF, tag="htr")
                    nc.tensor.transpose(tps[:], h2b[:, kc * 128:(kc + 1) * 128], ident[:])
                    nc.vector.tensor_copy(h2T_all[:, kc, t * 128:(t + 1) * 128], tps[:])

            # c1^T = relu(wc1'^T h2 + bc1') ; then c2 = relu(c1 @ wc2 + bc2)
            c1T = h2T_pool.tile([128, DKC, RPC], BF)
            for fc in range(DKC):
                for rc in range(RPC // 512):
                    cps = psC1.tile([128, 512], F32, tag="c1ps")
                    for kc in range(DKC):
                        nc.tensor.matmul(cps[:], wc1_sb[:, kc, fc * 128:(fc + 1) * 128],
                                         h2T_all[:, kc, rc * 512:(rc + 1) * 512],
                                         start=(kc == 0), stop=(kc == DKC - 1))
                    nc.scalar.activation(c1T[:, fc, rc * 512:(rc + 1) * 512], cps[:],
                                         AF.Relu, bias=bc1_sb[:, fc:fc + 1], scale=1.0)
            for rc in range(RPC // 512):
                c2ps = psC2.tile([1, 512], F32, tag="c2ps")
                for kc in range(DKC):
                    nc.tensor.matmul(c2ps[:], wc2_sb[:, kc:kc + 1],
                                     c1T[:, kc, rc * 512:(rc + 1) * 512],
                                     start=(kc == 0), stop=(kc == DKC - 1))
                nc.scalar.activation(c2_sb[0:1, rc * 512:(rc + 1) * 512], c2ps[:],
                                     AF.Relu, bias=bc2_sb[0:1, :], scale=1.0)
            nc.sync.dma_start(out=c_ag_in.ap().rearrange("(o n) -> o n", o=1),
                              in_=c2_sb[0:1, :])
            nc.gpsimd.collective_compute(
                "AllGather", OP.bypass,
                ins=[c_ag_in.ap()], outs=[c_ag_out.ap()], replica_groups=RG)

            # final: out = relu(c @ wl1 + bl1) @ wl2 + bl2 (redundant on all cores)
            cT = rowC.tile([128, 2, NC, B], F32, tag="cT")
            for e in range(2):
                nc.sync.dma_start(
                    out=cT[:, e, :, :],
                    in_=bass.AP(tensor=c_ag_out.ap().tensor, offset=e * 128,
                                ap=[[1, 128], [RPC, NC], [LPC, B]]))
            cTb = rowC.tile([128, 2, NC, B], BF, tag="cTb")
            nc.vector.tensor_copy(cTb[:], cT[:])
            bl1_bc = wC_pool.tile([B, L], F32)
            nc.sync.dma_start(out=bl1_bc[:], in_=bcast(bl1, B, L))
            bl2_bc = wC_pool.tile([B, OUT], F32)
            nc.sync.dma_start(out=bl2_bc[:], in_=bcast(bl2, B, OUT))
            c1f = rowC.tile([B, L], BF, tag="c1f")
            for oc in range(L // 512):
                wl1_oc = wl1_pool.tile([128, L // 128, 512], BF, tag="wl1oc")
                _delay(nc.sync.dma_start(
                    out=wl1_oc[:],
                    in_=bass.AP(tensor=wl1.ap().tensor, offset=oc * 512,
                                ap=[[L, 128], [128 * L, L // 128], [1, 512]])))
                fps = psFin.tile([B, 512], F32, tag="finps")
                for kc in range(L // 128):
                    nc.tensor.matmul(fps[:], cTb[:, kc % 2, kc // 2, :],
                                     wl1_oc[:, kc, :],
                                     start=(kc == 0), stop=(kc == L // 128 - 1))
                tmp = rowC.tile([B, 512], F32, tag="fintmp")
                nc.vector.tensor_add(tmp[:], fps[:], bl1_bc[:, oc * 512:(oc + 1) * 512])
                nc.vector.tensor_scalar_max(c1f[:, oc * 512:(oc + 1) * 512], tmp[:], 0.0)
            c1fT = rowC.tile([128, L // 128, B], BF, tag="c1fT")
            for j in range(L // 128):
                tps = psTrC.tile([128, B], BF, tag="htr")
                nc.tensor.transpose(tps[:], c1f[0:B, j * 128:(j + 1) * 128], ident[0:B, 0:B])
                nc.vector.tensor_copy(c1fT[:, j, :], tps[:])
            ops = psFin.tile([B, OUT], F32, tag="finps")
            for kc in range(L // 128):
                nc.tensor.matmul(ops[:], c1fT[:, kc, :], wl2_sb[:, kc, :],
                                 start=(kc == 0), stop=(kc == L // 128 - 1))
            out_f = rowC.tile([B, OUT], F32, tag="out_f")
            nc.vector.tensor_add(out_f[:], ops[:], bl2_bc[:])
            nc.sync.dma_start(out=out.ap(), in_=out_f[:])

    nc.compile()
    return nc


def _to_bf16(a):
    return np.asarray(a, dtype=np.float32).astype(ml_dtypes.bfloat16)


def kernel(**inputs):
    from concourse.bass_utils import run_bass_kernel_spmd

    if "nc" not in _CACHE:
        _CACHE["nc"] = _build_nc()
    nc = _CACHE["nc"]

    x = np.asarray(inputs["x"], dtype=np.float32).reshape(N, D)
    isq = 1.0 / math.sqrt(DH)
    gamma_np = np.asarray(inputs["gamma"], dtype=np.float32)
    beta_np = np.asarray(inputs["beta"], dtype=np.float32)
    wc1_np = np.asarray(inputs["wc1"], dtype=np.float32)
    bc1_np = np.asarray(inputs["bc1"], dtype=np.float32)
    # fold LN2's gamma/beta into the c1 projection (h2 feeds only this matmul)
    wc1_f = gamma_np[:, None] * wc1_np
    bc1_f = bc1_np + beta_np @ wc1_np

    xT_bf = np.ascontiguousarray(_to_bf16(x).T)
    shared = dict(
        xT=xT_bf,
        wff=_to_bf16(inputs["wff"]),
        bff=np.asarray(inputs["bff"], np.float32),
        gamma=_to_bf16(gamma_np), beta=_to_bf16(beta_np),
        wc1=_to_bf16(wc1_f), bc1=bc1_f.astype(np.float32),
        wc2=_to_bf16(np.asarray(inputs["wc2"]).reshape(D)),
        bc2=np.asarray(inputs["bc2"], np.float32).reshape(1),
        wl1=_to_bf16(inputs["wl1"]), bl1=np.asarray(inputs["bl1"], np.float32),
        wl2=_to_bf16(inputs["wl2"]), bl2=np.asarray(inputs["bl2"], np.float32),
    )
    wq = np.asarray(inputs["wq"], np.float32) * isq
    bq = np.asarray(inputs["bq"], np.float32) * isq
    wk = np.asarray(inputs["wk"], np.float32)
    bk = np.asarray(inputs["bk"], np.float32)
    wv = np.asarray(inputs["wv"], np.float32)
    bv = np.asarray(inputs["bv"], np.float32)

    in_maps = []
    for i in range(NC):
        sl = slice(i * DH, (i + 1) * DH)
        wqkv_i = np.stack([wq[:, sl], wk[:, sl], wv[:, sl]])
        bqkv_i = np.stack([bq[sl], bk[sl], bv[sl]])
        # rows this core owns after the A2A: for each batch b, L-positions
        # [i*LPC, (i+1)*LPC) -> 8 row-tiles of 128 = (b, e) pairs
        xr = np.concatenate([
            x[b * L + i * LPC: b * L + (i + 1) * LPC, :] for b in range(B)
        ])  # [RPC, D] ordered (b, l-within-block)
        in_maps.append(dict(
            shared,
            wqkv=_to_bf16(wqkv_i),
            bqkv=bqkv_i.astype(np.float32),
            xrows=_to_bf16(xr),
        ))

    res = run_bass_kernel_spmd(nc, in_maps, core_ids=list(range(NC)))
    return np.asarray(res.results[0]["out"], dtype=np.float32)
